# revision 1
# baseline (speedup 1.0000x reference)
"""Multi-head attention (B=4, N=2048, C=1024, H=16) on 8 trn2 NeuronCores.

Sharding: data-parallel over batch (4) x tensor-parallel over heads (2).
Core c handles batch c//2, heads [8*(c%2), 8*(c%2)+8). Each core computes a
partial output projection (contraction over its 512 channels); the host sums
core pairs and adds the projection bias.

Device-side math per core (n=2048 tokens, cp=512 channels, 8 heads, hd=64):
  qT/kT = (w @ x^T) in transposed layout [c', n]; v in natural layout [n, c']
  augmented with a ones column per head (gives the softmax denominator for
  free as row 64 of the attn@V matmul). Scores are computed transposed
  [k, q] per head, exp on ScalarE (no max subtraction; logits are bounded),
  mask applied as a bf16 multiply on VectorE, attn@V + denominator on
  TensorE, normalization via a rank-1 ones x dinv broadcast matmul, then the
  output projection. Matmuls run in fp32r (full PE rate for free dim >= 256).
"""

import os
import sys

for p in ("/opt/trn_rl_repo", "/root/.axon_site/_ro/trn_rl_repo"):
    if os.path.isdir(p) and p not in sys.path:
        sys.path.insert(0, p)

import ml_dtypes
import numpy as np

import concourse.bacc as bacc
import concourse.tile as tile
from concourse import mybir
from concourse.bass_utils import run_bass_kernel_spmd

FP = mybir.dt.float32
FR = mybir.dt.float32r
BF = mybir.dt.bfloat16
EXP = mybir.ActivationFunctionType.Exp

DIM = 1024
NUM_HEADS = 16
HEAD_DIM = 64
SCALE = HEAD_DIM ** -0.5
B, N = 4, 2048
NCORES = 8


def build_attention(n=N, c=DIM, cp=DIM // 2, hd=HEAD_DIM, scale=SCALE):
    """Emit the per-core program. All cores run the same code (SPMD)."""
    hpc = cp // hd          # heads on this core
    CB = c // 128           # contraction blocks for QKV
    MB = cp // 128          # c' blocks (q/k transposed layout)
    NB = n // 128           # token blocks
    QC = n // 512
    QW = min(1024, n)       # phase-2 q chunk width
    QH = n // QW            # q chunks (phase-2 outer loop)
    hd1 = hd + 1            # v augmented with a ones column -> denominator

    nc = bacc.Bacc("TRN2", target_bir_lowering=False, debug=False)

    xT = nc.dram_tensor("xT", [c, n], FR, kind="ExternalInput").ap()
    wqT = nc.dram_tensor("wqT", [c, cp], FR, kind="ExternalInput").ap()
    wkT = nc.dram_tensor("wkT", [c, cp], FR, kind="ExternalInput").ap()
    wvT = nc.dram_tensor("wvT", [c, cp], FR, kind="ExternalInput").ap()
    wpT = nc.dram_tensor("wpT", [cp, c], FR, kind="ExternalInput").ap()
    maskT = nc.dram_tensor("maskT", [n, n], BF, kind="ExternalInput").ap()
    out = nc.dram_tensor("out", [n, c], FP, kind="ExternalOutput").ap()

    with tile.TileContext(nc) as tc:
        with (
            tc.tile_pool(name="persist", bufs=1) as pers,
            tc.tile_pool(name="d_pool", bufs=1) as dpool,
        ):
            qT_sb = pers.tile([128, MB, n], FR, tag="qT")
            kT_sb = pers.tile([128, MB, n], FR, tag="kT")
            vaug_sb = pers.tile([128, NB, hpc * hd1], BF, tag="vaug")
            d_sb = dpool.tile([hpc, n], FP, tag="dsum")

            # ---------------- Phase 1: QKV projections ----------------
            with (
                tc.tile_pool(name="xt", bufs=1) as xpool,
                tc.tile_pool(name="w", bufs=2) as wpool,
                tc.tile_pool(name="ps_qkv", bufs=6, space="PSUM") as pq,
            ):
                xT_sb = xpool.tile([128, CB, n], FR, tag="xT")
                for cb in range(CB):
                    nc.sync.dma_start(
                        xT_sb[:, cb, :],
                        xT.rearrange("(cb p) n -> p cb n", p=128)[:, cb, :],
                    )
                w_aps = {"q": wqT, "k": wkT, "v": wvT}
                w_sb = {}
                for wn in ("q", "k", "v"):
                    wt = wpool.tile([128, CB, cp], FR, tag="w")
                    nc.sync.dma_start(
                        wt, w_aps[wn].rearrange("(cb p) m -> p cb m", p=128)
                    )
                    w_sb[wn] = wt

                # qT/kT: out [c' block, n] ; lhsT = w chunk, rhs = xT chunk
                for wn, dst in (("q", qT_sb), ("k", kT_sb)):
                    for mb in range(MB):
                        for qc in range(QC):
                            pt = pq.tile([128, 512], FP, tag="psqkv")
                            for cb in range(CB):
                                nc.tensor.matmul(
                                    pt,
                                    lhsT=w_sb[wn][:, cb, mb * 128:(mb + 1) * 128],
                                    rhs=xT_sb[:, cb, qc * 512:(qc + 1) * 512],
                                    start=(cb == 0),
                                    stop=(cb == CB - 1),
                                )
                            nc.vector.tensor_copy(
                                dst[:, mb, qc * 512:(qc + 1) * 512], pt
                            )
                # v: natural layout [n block, c'] ; lhsT = xT chunk, rhs = wvT
                for nb in range(NB):
                    pt = pq.tile([128, cp], FP, tag="psqkv")
                    for cb in range(CB):
                        nc.tensor.matmul(
                            pt,
                            lhsT=xT_sb[:, cb, nb * 128:(nb + 1) * 128],
                            rhs=w_sb["v"][:, cb, :],
                            start=(cb == 0),
                            stop=(cb == CB - 1),
                        )
                    dst3 = vaug_sb[:, nb, :].rearrange("p (h e) -> p h e", e=hd1)
                    nc.vector.tensor_copy(
                        dst3[:, :, 0:hd],
                        pt.rearrange("p (h e) -> p h e", e=hd),
                    )
                    nc.vector.memset(dst3[:, :, hd:hd1], 1.0)

            # ---------------- Phase 2: scores / softmax / attn@V ------------
            with (
                tc.tile_pool(name="aoT", bufs=1) as aop,
                tc.tile_pool(name="wp", bufs=1) as wppool,
            ):
                aoT_sb = aop.tile([128, MB, n], FR, tag="aoT")
                wp_sb = wppool.tile([128, MB, c], FR, tag="wp")
                with (
                    tc.tile_pool(name="mask", bufs=1) as mpool,
                    tc.tile_pool(name="ps_sc", bufs=3, space="PSUM") as psc,
                    tc.tile_pool(name="ps_ao", bufs=1, space="PSUM") as pao,
                    tc.tile_pool(name="s_exp", bufs=6) as sep,
                    tc.tile_pool(name="s_m", bufs=6) as smp,
                ):
                    for qh in range(QH):
                        qo = qh * QW
                        mk = mpool.tile([128, NB, QW], BF, tag="maskT")
                        for kb in range(NB):
                            nc.sync.dma_start(
                                mk[:, kb, :],
                                maskT.rearrange("(kb p) q -> p kb q", p=128)[
                                    :, kb, qo:qo + QW
                                ],
                            )
                        if qh == 0:
                            # preload the projection weights behind the first
                            # mask chunk so the tail never waits on this DMA
                            nc.sync.dma_start(
                                wp_sb,
                                wpT.rearrange("(mb p) co -> p mb co", p=128),
                            )
                        # software-pipelined over units (h, kb): emit the
                        # scores matmuls LOOK units ahead of exp/mask/attn@V
                        # so the PE FIFO never drains at head transitions.
                        units = [(h, kb) for h in range(hpc) for kb in range(NB)]
                        LOOK = 2
                        sc_map = {}
                        ao_map = {}
                        for idx in range(len(units) + LOOK):
                            if idx < len(units):
                                h, kb = units[idx]
                                po = (h % 2) * hd
                                hb = h // 2
                                sc_t = psc.tile([128, QW], FP, tag="sc")
                                sc_map[idx] = sc_t
                                sc = sc_t
                                for qs in range(QW // 512):
                                    nc.tensor.matmul(
                                        sc[:, qs * 512:(qs + 1) * 512],
                                        lhsT=kT_sb[po:po + hd, hb, kb * 128:(kb + 1) * 128],
                                        rhs=qT_sb[po:po + hd, hb, qo + qs * 512:qo + (qs + 1) * 512],
                                        start=True,
                                        stop=True,
                                    )
                            j = idx - LOOK
                            if j < 0:
                                continue
                            h, kb = units[j]
                            po = (h % 2) * hd
                            hb = h // 2
                            sc = sc_map.pop(j)
                            se = sep.tile([128, QW], BF, tag="se")
                            nc.scalar.activation(se, sc, EXP, scale=scale)
                            sm = smp.tile([128, QW], BF, tag="sm")
                            nc.vector.tensor_mul(sm, se, mk[:, kb, :])
                            if kb == 0:
                                ao_new = pao.tile([hd1, QW], FP, tag="ao")
                                ao_map[h] = ao_new
                            ao = ao_map[h]
                            for qs in range(QW // 512):
                                nc.tensor.matmul(
                                    ao[:, qs * 512:(qs + 1) * 512],
                                    lhsT=vaug_sb[:, kb, h * hd1:(h + 1) * hd1],
                                    rhs=sm[:, qs * 512:(qs + 1) * 512],
                                    start=(kb == 0),
                                    stop=(kb == NB - 1),
                                )
                            if kb == NB - 1:
                                nc.vector.tensor_copy(
                                    aoT_sb[po:po + hd, hb, qo:qo + QW],
                                    ao[0:hd, :],
                                )
                                # D row: PSUM partition 64 -> partition-0 SBUF
                                # tile (aligned start partitions), then DMA
                                # into row h of the batch tile on the ACT
                                # queue (keeps it off the bulk-DMA queue).
                                dtmp = dpool.tile([1, QW], FP, tag="dtmp")
                                nc.vector.tensor_copy(dtmp, ao[hd:hd1, :])
                                nc.sync.dma_start(
                                    d_sb[h:h + 1, qo:qo + QW], dtmp
                                )
                                del ao_map[h]

                # ---- normalization + output projection ----
                with tc.tile_pool(name="dinv", bufs=2) as dip:
                    dinv = dip.tile([hpc, n], FP, tag="dinv")
                    ones_raw = dip.tile([1, hd], FP, tag="ones_raw")
                    nc.vector.memset(ones_raw, 1.0)
                    ones_sb = dip.tile([1, hd], FR, tag="ones")
                    nc.vector.tensor_copy(ones_sb, ones_raw)
                    # ~51-ULP reciprocal is plenty for softmax denominators
                    nc.vector.reciprocal_approx_fast(dinv, d_sb)
                    with tc.tile_pool(name="ps_bc", bufs=2, space="PSUM") as pbc:
                        for h in range(hpc):
                            po = (h % 2) * hd
                            hb = h // 2
                            # stage dinv row h at partition 0 (via DMA: DVE
                            # and PE need 0/32/64-aligned start partitions),
                            # then broadcast as ones[hd,1] x d0[1,n] matmul.
                            d0 = dip.tile([1, n], FR, tag="d0")
                            nc.scalar.dma_start(d0, dinv[h:h + 1, :].bitcast(FR))
                            bc = pbc.tile([hd, n], FP, tag="bc")
                            for qc in range(QC):
                                nc.tensor.matmul(
                                    bc[:, qc * 512:(qc + 1) * 512],
                                    lhsT=ones_sb,
                                    rhs=d0[:, qc * 512:(qc + 1) * 512],
                                    start=True,
                                    stop=True,
                                )
                            nc.vector.tensor_mul(
                                aoT_sb[po:po + hd, hb, :],
                                aoT_sb[po:po + hd, hb, :],
                                bc,
                            )

                    with (
                        tc.tile_pool(name="ps_o", bufs=4, space="PSUM") as pso,
                        tc.tile_pool(name="osb", bufs=3) as osp,
                    ):
                        for nb in range(NB):
                            ot = osp.tile([128, c], FP, tag="ot")
                            for co in range(c // 512):
                                pt = pso.tile([128, 512], FP, tag="pso")
                                for mb in range(MB):
                                    nc.tensor.matmul(
                                        pt,
                                        lhsT=aoT_sb[:, mb, nb * 128:(nb + 1) * 128],
                                        rhs=wp_sb[:, mb, co * 512:(co + 1) * 512],
                                        start=(mb == 0),
                                        stop=(mb == MB - 1),
                                    )
                                nc.vector.tensor_copy(
                                    ot[:, co * 512:(co + 1) * 512], pt
                                )
                            nc.sync.dma_start(
                                out.rearrange("(nb p) co -> p nb co", p=128)[:, nb, :],
                                ot,
                            )
    nc.compile()
    return nc


def make_in_maps(x, mask, wq, wk, wv, wp):
    """Host-side sharding: per-core input dict."""
    bf16 = ml_dtypes.bfloat16
    in_maps = []
    for core in range(NCORES):
        b = core // 2
        g = core % 2
        cs = slice(g * 512, (g + 1) * 512)
        in_maps.append({
            "xT": np.ascontiguousarray(x[b].T).astype(np.float32, copy=False),
            "wqT": np.ascontiguousarray(wq[cs, :].T),
            "wkT": np.ascontiguousarray(wk[cs, :].T),
            "wvT": np.ascontiguousarray(wv[cs, :].T),
            "wpT": np.ascontiguousarray(wp[:, cs].T),
            "maskT": np.ascontiguousarray(mask[b].T).astype(bf16),
        })
    return in_maps


_NC_CACHE = {}


def _get_nc():
    if "nc" not in _NC_CACHE:
        _NC_CACHE["nc"] = build_attention()
    return _NC_CACHE["nc"]


def kernel(x, mask, wq, wk, wv, wp, bp, _trace=False, _trace_kwargs=None):
    x = np.asarray(x, dtype=np.float32)
    mask = np.asarray(mask)
    wq = np.asarray(wq, dtype=np.float32)
    wk = np.asarray(wk, dtype=np.float32)
    wv = np.asarray(wv, dtype=np.float32)
    wp = np.asarray(wp, dtype=np.float32)
    bp = np.asarray(bp, dtype=np.float32)

    nc = _get_nc()
    in_maps = make_in_maps(x, mask, wq, wk, wv, wp)
    kw = {}
    if _trace:
        kw = {"trace": True, **(_trace_kwargs or {})}
    res = run_bass_kernel_spmd(nc, in_maps, list(range(NCORES)), **kw)
    outs = [np.asarray(r["out"], dtype=np.float32) for r in res.results]
    full = np.empty((B, N, DIM), dtype=np.float32)
    for b in range(B):
        full[b] = outs[2 * b] + outs[2 * b + 1] + bp[None, :]
    if _trace:
        return full, res
    return full


if __name__ == "__main__":
    nc = build_attention()
    print("built ok")

